# revision 1
# baseline (speedup 1.0000x reference)
"""Distributed Trainium2 kernel for the dense-graph GNN layer.

Math: with xn = x/||x|| (rows), G = xn@xn.T, d = rsqrt(G@1),
out = (diag(d) G diag(d) x) W.  The N x N Gram matrix is never needed:
  G @ 1        = xn @ t,            t = colsum(xn)            [D]
  diag(d) G diag(d) x = f * (x @ z),  z = x.T @ diag(f) @ x   [D, D]
  f_i = d_i / ||x_i||   (combines both scalings; z is symmetric)
  out = f * (x @ (z @ W))
So each core processes its 1024-row shard with O(N D^2) flops and the only
cross-core traffic is an AllGather of a [1,256] colsum partial and an
AllReduce of the [256,256] (z @ W) partial.
"""

import os
import sys

import numpy as np

for _p in ("/opt/trn_rl_repo", "/root/.axon_site/_ro/trn_rl_repo"):
    if os.path.isdir(_p) and _p not in sys.path:
        sys.path.insert(0, _p)

import concourse.bacc as bacc
import concourse.mybir as mybir
import concourse.tile as tile
import concourse.masks as masks
from concourse import bass_utils

R = 8                 # cores
N, D = 8192, 256
NL = N // R           # 1024 rows per core
P = 128
T = NL // P           # 8 row tiles per core
F32 = mybir.dt.float32
BF16 = mybir.dt.bfloat16
AF = mybir.ActivationFunctionType
ALU = mybir.AluOpType

_cache = {}


def _program(tc, x, W, out):
    nc = tc.nc
    rg = [list(range(R))]
    if True:
        with (
            tc.tile_pool(name="persist", bufs=1) as pp,
            tc.tile_pool(name="work", bufs=3) as wp,
            tc.tile_pool(name="psum", bufs=1, space="PSUM") as psp,
            tc.tile_pool(name="psumw", bufs=4, space="PSUM") as psw,
            tc.tile_pool(name="dram", bufs=1, space="DRAM") as dp,
        ):
            x_all = pp.tile([P, T * D], F32)      # row tile i at [:, i*D:(i+1)*D]
            xb_all = pp.tile([P, T * D], BF16)    # bf16 copy of x
            g_all = pp.tile([P, T * D], BF16)     # f * x (bf16)
            xT_all = pp.tile([P, 2 * NL], BF16)   # x.T chunk c at [:, c*NL + i*P]
            W_sb = pp.tile([P, 2 * D], F32)       # W k-chunk kc at [:, kc*D]
            Wb_sb = pp.tile([P, 2 * D], BF16)
            zw_sb = pp.tile([P, 2 * D], BF16)     # zw a-chunk ka at [:, ka*D]
            zT_sb = pp.tile([P, 2 * D], BF16)

            ss = pp.tile([P, T], F32)
            invn = pp.tile([P, T], F32)
            nrm = pp.tile([P, T], F32)
            stl = pp.tile([P, T], F32)
            s_t = pp.tile([P, T], F32)
            sq_s = pp.tile([P, T], F32)
            dd = pp.tile([P, T], F32)
            f_t = pp.tile([P, T], F32)

            ident = pp.tile([P, P], F32)
            masks.make_identity(nc, ident[:])
            ones8 = pp.tile([8, P], F32)
            nc.gpsimd.memset(ones8[:], 1.0)

            cc_t_in = dp.tile([1, D], F32)
            cc_t_out = dp.tile([R, D], F32)
            cc_zw_in = dp.tile([2 * P, D], BF16)
            cc_zw_out = dp.tile([2 * P, D], BF16)

            for kc in range(2):
                nc.sync.dma_start(W_sb[:, kc * D:(kc + 1) * D], W[kc * P:(kc + 1) * P, :])
            nc.vector.tensor_copy(Wb_sb[:], W_sb[:])

            # ---- phase A: load shard, row norms, colsum(xn) partial ----
            for i in range(T):
                xs = x_all[:, i * D:(i + 1) * D]
                nc.sync.dma_start(xs, x[i * P:(i + 1) * P, :])
                scr = wp.tile([P, D], F32, tag="scr", name=f"scr{i}")
                nc.scalar.activation(scr[:], xs, AF.Square, accum_out=ss[:, i:i + 1])
                nc.vector.tensor_copy(xb_all[:, i * D:(i + 1) * D], xs)
            nc.scalar.activation(nrm[:], ss[:], AF.Sqrt)
            nc.vector.reciprocal(invn[:], nrm[:])

            psum_t = psp.tile([1, D], F32, name="psum_t")
            for i in range(T):
                nc.tensor.matmul(
                    psum_t[:], lhsT=invn[:, i:i + 1], rhs=x_all[:, i * D:(i + 1) * D],
                    start=(i == 0), stop=(i == T - 1),
                )
            t_sb = pp.tile([1, D], F32)
            nc.vector.tensor_copy(t_sb[:], psum_t[:])
            nc.sync.dma_start(cc_t_in[:], t_sb[:])
            nc.gpsimd.collective_compute(
                "AllGather", ALU.bypass, replica_groups=rg,
                ins=[cc_t_in.opt()], outs=[cc_t_out.opt()],
            )

            # x.T via PE transposes (independent of the collective -> overlaps it)
            for i in range(T):
                for c in range(2):
                    pt = psw.tile([P, P], F32, tag="pw", name=f"pt{i}_{c}")
                    nc.tensor.transpose(
                        pt[:], x_all[:, i * D + c * P: i * D + (c + 1) * P], ident[:]
                    )
                    nc.vector.tensor_copy(xT_all[:, c * NL + i * P: c * NL + (i + 1) * P], pt[:])

            tg_sb = pp.tile([8, D], F32)
            nc.sync.dma_start(tg_sb[:], cc_t_out[:])
            # sum the 8 rank partials AND broadcast to 128 partitions in one matmul
            psum_tb = psp.tile([P, D], F32, name="psum_tb")
            nc.tensor.matmul(psum_tb[:], lhsT=ones8[:], rhs=tg_sb[:], start=True, stop=True)

            # ---- phase B: degrees, f, g = f*x, zT partial, zw partial ----
            tb_sb = pp.tile([P, D], F32)
            nc.vector.tensor_copy(tb_sb[:], psum_tb[:])
            big_scr = pp.tile([P, T * D], F32)
            t_ap = tb_sb[:]
            from concourse.bass_types import AP as _AP
            t_rep = _AP(t_ap.tensor, t_ap.offset, [t_ap.ap[0], [0, T], t_ap.ap[1]])
            x3 = x_all[:].rearrange("p (t d) -> p t d", t=T)
            s3 = big_scr[:].rearrange("p (t d) -> p t d", t=T)
            nc.vector.tensor_mul(s3, x3, t_rep)
            nc.vector.tensor_reduce(stl[:], s3, axis=mybir.AxisListType.X, op=ALU.add)
            nc.vector.tensor_mul(s_t[:], stl[:], invn[:])       # s = rowsum * invn
            nc.scalar.activation(sq_s[:], s_t[:], AF.Sqrt)
            nc.vector.reciprocal(dd[:], sq_s[:])                # d = rsqrt(s)
            nc.vector.tensor_mul(f_t[:], dd[:], invn[:])        # f = d * invn
            for i in range(T):
                nc.scalar.mul(g_all[:, i * D:(i + 1) * D], x_all[:, i * D:(i + 1) * D],
                              f_t[:, i:i + 1])

            psum_zT0 = psp.tile([P, D], F32, name="pzT0")
            psum_zT1 = psp.tile([P, D], F32, name="pzT1")
            for i in range(T):
                for c, pz in ((0, psum_zT0), (1, psum_zT1)):
                    nc.tensor.matmul(
                        pz[:], lhsT=xb_all[:, i * D + c * P: i * D + (c + 1) * P],
                        rhs=g_all[:, i * D:(i + 1) * D],
                        start=(i == 0), stop=(i == T - 1),
                    )
            for c, pz in ((0, psum_zT0), (1, psum_zT1)):
                nc.vector.tensor_copy(zT_sb[:, c * D:(c + 1) * D], pz[:])


            # zw partial = z_p @ W (fold the W GEMM before the collective)
            for m in range(2):
                pzw = psw.tile([P, D], F32, tag="pw", name=f"pzw{m}")
                for kc in range(2):
                    nc.tensor.matmul(
                        pzw[:], lhsT=zT_sb[:, kc * D + m * P: kc * D + (m + 1) * P],
                        rhs=Wb_sb[:, kc * D:(kc + 1) * D],
                        start=(kc == 0), stop=(kc == 1),
                    )
                zwp_sb = wp.tile([P, D], BF16, tag="zwp", name=f"zwp{m}")
                nc.vector.tensor_copy(zwp_sb[:], pzw[:])
                nc.sync.dma_start(cc_zw_in[m * P:(m + 1) * P, :], zwp_sb[:])
            nc.gpsimd.collective_compute(
                "AllReduce", ALU.add, replica_groups=rg,
                ins=[cc_zw_in.opt()], outs=[cc_zw_out.opt()],
            )
            for ka in range(2):
                nc.sync.dma_start(zw_sb[:, ka * D:(ka + 1) * D], cc_zw_out[ka * P:(ka + 1) * P, :])

            # ---- phase C: out = f * (x @ zw) ----
            for i in range(T):
                po = psw.tile([P, D], F32, tag="pw", name=f"po{i}")
                for ka in range(2):
                    nc.tensor.matmul(
                        po[:], lhsT=xT_all[:, ka * NL + i * P: ka * NL + (i + 1) * P],
                        rhs=zw_sb[:, ka * D:(ka + 1) * D],
                        start=(ka == 0), stop=(ka == 1),
                    )
                o_sb = wp.tile([P, D], F32, tag="osb", name=f"osb{i}")
                nc.scalar.mul(o_sb[:], po[:], f_t[:, i:i + 1])
                nc.sync.dma_start(out[i * P:(i + 1) * P, :], o_sb[:])


def _build():
    nc = bacc.Bacc("TRN2", target_bir_lowering=False, debug=False, num_devices=R)
    x = nc.dram_tensor("x", [NL, D], F32, kind="ExternalInput")
    W = nc.dram_tensor("W", [D, D], F32, kind="ExternalInput")
    out = nc.dram_tensor("out", [NL, D], F32, kind="ExternalOutput")
    with tile.TileContext(nc) as tc:
        _program(tc, x.ap() if hasattr(x, "ap") else x, W.ap() if hasattr(W, "ap") else W, out.ap() if hasattr(out, "ap") else out)
    nc.finalize()
    return nc


def _run(inputs, trace=False):
    if "nc" not in _cache:
        _cache["nc"] = _build()
    nc = _cache["nc"]
    x = np.ascontiguousarray(inputs["x"], dtype=np.float32)
    W = np.ascontiguousarray(inputs["W"], dtype=np.float32)
    in_maps = [{"x": x[r * NL:(r + 1) * NL], "W": W} for r in range(R)]
    res = bass_utils.run_bass_kernel_spmd(
        nc, in_maps, core_ids=list(range(R)), trace=trace,
    )
    out = np.concatenate([res.results[r]["out"] for r in range(R)], axis=0)
    return out, res


def kernel(**inputs) -> np.ndarray:
    out, _ = _run(inputs, trace=False)
    return out



# revision 4
# speedup vs baseline: 1.1559x; 1.1559x over previous
"""Replicated (collective-free) Trainium2 kernel for the dense-graph GNN layer.

Math: with xn = x/||x|| (rows), G = xn@xn.T, d = rsqrt(G@1),
out = (diag(d) G diag(d) x) W.  The N x N Gram matrix is never formed:
  t = colsum(xn)                      [D]
  s = xn @ t      (degrees)           [N]
  d = rsqrt(s),  w = d * ||x||        [N]
  z = xn.T @ diag(w) @ xn             [D, D]   (symmetric)
  out = diag(d) @ xn @ (z @ W)

Distribution: NO collectives.  Each of the 8 cores reads the FULL x from
HBM (8 MB, streamed and fully overlapped with compute) and redundantly
computes t, s, d, w, z, zw.  Core r is fed a row-ROTATED copy of x
(rows r*1024.. wrapped to the front) so the program is rank-agnostic:
every core emits the output for "its" first 1024 rows.  This removes the
AllGather + AllReduce + CC barrier that dominated the sharded version.
"""

import os
import sys

import numpy as np

for _p in ("/opt/trn_rl_repo", "/root/.axon_site/_ro/trn_rl_repo"):
    if os.path.isdir(_p) and _p not in sys.path:
        sys.path.insert(0, _p)

import concourse.bacc as bacc
import concourse.mybir as mybir
import concourse.tile as tile
import concourse.masks as masks
from concourse import bass_utils
from concourse.bass_types import AP as _AP

R = 8                 # cores
N, D = 8192, 256
NL = N // R           # 1024 output rows per core
P = 128
NT = N // P           # 64 row tiles streamed per core
CH = 4                # tiles per input DMA chunk
NCH = NT // CH        # 16 chunks
LT = NL // P          # 8 local (output) tiles = tiles 0..7 of the rotated view
GRP = 8               # tiles per s/g/z pipeline group
NG = NT // GRP
F32 = mybir.dt.float32
BF16 = mybir.dt.bfloat16
AF = mybir.ActivationFunctionType
ALU = mybir.AluOpType

_cache = {}


def _program(tc, x, W, out):
    nc = tc.nc
    with (
        tc.tile_pool(name="persist", bufs=1) as pp,
        tc.tile_pool(name="work", bufs=4) as wp,
        tc.tile_pool(name="psacc", bufs=1, space="PSUM") as psp,
        tc.tile_pool(name="pswork", bufs=2, space="PSUM") as psw,
    ):
        xnb = pp.tile([P, NT * D], BF16)      # normalized rows, tile i at [:, i*D:]
        g_all = pp.tile([P, NT * D], BF16)    # w-scaled normalized rows
        xnbT = pp.tile([P, 2 * NL], BF16)     # local xn.T; chunk h, tile i at h*NL+i*P
        scr3 = pp.tile([P, GRP * D], BF16)    # s-dot scratch (reused per group)
        ss = pp.tile([P, NT], F32)            # sum of squares per row
        nrm = pp.tile([P, NT], F32)
        invn = pp.tile([P, NT], F32)
        s_all = pp.tile([P, NT], F32)
        rec = pp.tile([P, NT], F32)
        d_all = pp.tile([P, NT], F32)
        w_all = pp.tile([P, NT], F32)
        t16 = pp.tile([1, D], BF16)
        tb16 = pp.tile([P, D], BF16)
        W_sb = pp.tile([P, 2 * D], F32)
        Wb = pp.tile([P, 2 * D], BF16)
        zb = pp.tile([P, 2 * D], BF16)
        zwb = pp.tile([P, 2 * D], BF16)
        onesP = pp.tile([P, 1], BF16)
        ones1 = pp.tile([1, P], BF16)
        identb = pp.tile([P, P], BF16)

        nc.gpsimd.memset(onesP[:], 1.0)
        nc.gpsimd.memset(ones1[:], 1.0)
        masks.make_identity(nc, identb[:])

        psum_t = psp.tile([1, D], F32, padded_shape=[1, 512])
        pz0 = psp.tile([P, D], F32, padded_shape=[P, 512])
        pz1 = psp.tile([P, D], F32, padded_shape=[P, 512])

        # ---- phase A: stream all of x; norms, xn (bf16), colsum(xn), local xn.T ----
        for c in range(NCH):
            xs = wp.tile([P, CH * D], F32, tag="xs", name=f"xs{c}")
            src = _AP(x.tensor, x.offset + c * CH * P * D,
                      [[D, P], [P * D, CH], [1, D]])
            nc.sync.dma_start(xs[:].rearrange("p (j d) -> p j d", j=CH), src)
            for j in range(CH):
                i = c * CH + j
                scr = wp.tile([P, D], F32, tag="scr", name=f"sq{i}")
                nc.scalar.activation(scr[:], xs[:, j * D:(j + 1) * D], AF.Square,
                                     accum_out=ss[:, i:i + 1])
            cs = slice(c * CH, (c + 1) * CH)
            nc.scalar.sqrt(nrm[:, cs], ss[:, cs])
            nc.vector.reciprocal(invn[:, cs], nrm[:, cs])
            for j in range(CH):
                i = c * CH + j
                nc.vector.tensor_scalar_mul(xnb[:, i * D:(i + 1) * D],
                                            xs[:, j * D:(j + 1) * D],
                                            invn[:, i:i + 1])
                nc.tensor.matmul(psum_t[:], lhsT=onesP[:],
                                 rhs=xnb[:, i * D:(i + 1) * D],
                                 start=(i == 0), stop=(i == NT - 1))
            if c * CH < LT:  # local tiles: build xn.T via PE transposes
                for j in range(CH):
                    i = c * CH + j
                    for h in range(2):
                        pt = psw.tile([P, P], BF16, tag="pt", name=f"pt{i}_{h}")
                        nc.tensor.transpose(
                            pt[:], xnb[:, i * D + h * P: i * D + (h + 1) * P],
                            identb[:])
                        nc.vector.tensor_copy(
                            xnbT[:, h * NL + i * P: h * NL + (i + 1) * P], pt[:])

        # W load on the scalar-engine DMA queue (off the x stream)
        for kc in range(2):
            nc.scalar.dma_start(W_sb[:, kc * D:(kc + 1) * D],
                                W[kc * P:(kc + 1) * P, :])
        nc.vector.tensor_copy(Wb[:], W_sb[:])

        # ---- t -> broadcast to all partitions ----
        nc.vector.tensor_copy(t16[:], psum_t[:])
        ptb = psw.tile([P, D], F32, tag="pw", name="ptb")
        nc.tensor.matmul(ptb[:], lhsT=ones1[:], rhs=t16[:], start=True, stop=True)
        nc.vector.tensor_copy(tb16[:], ptb[:])
        tb_ap = tb16[:]
        tb_rep = _AP(tb_ap.tensor, tb_ap.offset,
                     [tb_ap.ap[0], [0, GRP], tb_ap.ap[1]])

        # ---- phase B: degrees s, weights w, g = w*xn, z accumulation ----
        for gi in range(NG):
            gs = slice(gi * GRP, (gi + 1) * GRP)
            x3 = xnb[:, gi * GRP * D:(gi + 1) * GRP * D].rearrange(
                "p (t d) -> p t d", t=GRP)
            s3 = scr3[:].rearrange("p (t d) -> p t d", t=GRP)
            nc.vector.tensor_mul(s3, x3, tb_rep)
            nc.vector.tensor_reduce(s_all[:, gs], s3,
                                    axis=mybir.AxisListType.X, op=ALU.add)
            nc.vector.reciprocal(rec[:, gs], s_all[:, gs])
            nc.scalar.sqrt(d_all[:, gs], rec[:, gs])          # d = rsqrt(s)
            nc.vector.tensor_mul(w_all[:, gs], d_all[:, gs], nrm[:, gs])
            for j in range(GRP):
                i = gi * GRP + j
                gt = g_all[:, i * D:(i + 1) * D]
                if j % 2 == 0:
                    nc.scalar.mul(gt, xnb[:, i * D:(i + 1) * D], w_all[:, i:i + 1])
                else:
                    nc.vector.tensor_scalar_mul(gt, xnb[:, i * D:(i + 1) * D],
                                                w_all[:, i:i + 1])
                for h, pz in ((0, pz0), (1, pz1)):
                    nc.tensor.matmul(pz[:],
                                     lhsT=g_all[:, i * D + h * P: i * D + (h + 1) * P],
                                     rhs=xnb[:, i * D:(i + 1) * D],
                                     start=(i == 0), stop=(i == NT - 1))

        nc.vector.tensor_copy(zb[:, 0:D], pz0[:])
        nc.vector.tensor_copy(zb[:, D:2 * D], pz1[:])

        # ---- zw = z @ W ----
        for m in range(2):
            pzw = psw.tile([P, D], F32, tag="pw", name=f"pzw{m}")
            for h in range(2):
                nc.tensor.matmul(pzw[:],
                                 lhsT=zb[:, h * D + m * P: h * D + (m + 1) * P],
                                 rhs=Wb[:, h * D:(h + 1) * D],
                                 start=(h == 0), stop=(h == 1))
            nc.vector.tensor_copy(zwb[:, m * D:(m + 1) * D], pzw[:])

        # ---- phase C: out = diag(d) xn_local (zw) ----
        for i in range(LT):
            po = psw.tile([P, D], F32, tag="pw", name=f"po{i}")
            for h in range(2):
                nc.tensor.matmul(po[:],
                                 lhsT=xnbT[:, h * NL + i * P: h * NL + (i + 1) * P],
                                 rhs=zwb[:, h * D:(h + 1) * D],
                                 start=(h == 0), stop=(h == 1))
            o_sb = wp.tile([P, D], F32, tag="o", name=f"o{i}")
            nc.scalar.mul(o_sb[:], po[:], d_all[:, i:i + 1])
            nc.sync.dma_start(out[i * P:(i + 1) * P, :], o_sb[:])


def _build():
    nc = bacc.Bacc("TRN2", target_bir_lowering=False, debug=False, num_devices=R)
    x = nc.dram_tensor("x", [N, D], F32, kind="ExternalInput")
    W = nc.dram_tensor("W", [D, D], F32, kind="ExternalInput")
    out = nc.dram_tensor("out", [NL, D], F32, kind="ExternalOutput")
    with tile.TileContext(nc) as tc:
        _program(tc, x.ap() if hasattr(x, "ap") else x,
                 W.ap() if hasattr(W, "ap") else W,
                 out.ap() if hasattr(out, "ap") else out)
    nc.finalize()
    return nc


def _run(inputs, trace=False):
    if "nc" not in _cache:
        _cache["nc"] = _build()
    nc = _cache["nc"]
    x = np.ascontiguousarray(inputs["x"], dtype=np.float32)
    W = np.ascontiguousarray(inputs["W"], dtype=np.float32)
    in_maps = []
    for r in range(R):
        xr = np.concatenate([x[r * NL:], x[:r * NL]], axis=0) if r else x
        in_maps.append({"x": xr, "W": W})
    res = bass_utils.run_bass_kernel_spmd(
        nc, in_maps, core_ids=list(range(R)), trace=trace,
    )
    out = np.concatenate([res.results[r]["out"] for r in range(R)], axis=0)
    return out, res


def kernel(**inputs) -> np.ndarray:
    out, _ = _run(inputs, trace=False)
    return out
